# revision 26
# baseline (speedup 1.0000x reference)
"""Distance-modulated attention on 8 Trainium2 NeuronCores (Bass/Tile).

Sharding: core c handles batch b = c//2 and head-group hg = c%2 (8 of 16 heads).
Tensor-parallel over heads for QKV/out projections; per-batch distance matrix
is computed on-device per core. Host only slices/transposes (layout), concats,
and sums the two head-group partial outputs per batch (the TP all-reduce).

Per-core device pipeline (S=1024, E=1024, 8 heads x D=64):
  - cast inputs/weights fp32->bf16 on DVE/ACT/GPSIMD
  - d2 via K=5 fp32 matmul of [n_i,1,-2r] x [1,n_j,r]; dw' = 0.625/clip(sqrt(d2),1,30)
    computed as exp(-0.5*ln(clip(d2,1,900)) + ln(0.625)) on ACT (one table set)
  - QT = Wq'^T X^T, KT likewise (transposed layout), V = X Wv' (normal layout,
    bias via K=1 ones-row matmul), all bf16 MMs with fp32 PSUM accumulation
  - per head: S^T = K Q^T (K=64 bf16 MM) -> tmod = S^T * dw' (DVE fp32)
    -> E^T = exp(tmod) (ACT, bf16 out) -> U'^T = [V|1]^T E^T (PV matmul;
    row 64 = softmax denominator Z) -> per 128-row tile: PE-transpose U',
    rz = 1/Z (DVE reciprocal), O = U/Z (bf16), PE-transpose back to O^T;
    probs: PE-transpose E^T tiles, evict*rz to fp32 (DVE/ACT alternating), DMA
  - out = O^T.T Wo' + bo/2 (bf16 MM, bias via ones-row matmul)
"""
import numpy as np
import concourse.bass as bass
import concourse.tile as tile
import concourse.mybir as mybir
from concourse import bacc
from concourse.bass import ts
from concourse.bass_utils import run_bass_kernel_spmd
from concourse.masks import make_identity

F32 = mybir.dt.float32
BF16 = mybir.dt.bfloat16
AF = mybir.ActivationFunctionType
ALU = mybir.AluOpType

S = 1024          # sequence length
E = 1024          # embed dim
HG = 8            # heads per core
D = 64            # head dim
HD = HG * D       # 512, per-core projection width
NT = S // 128     # 8 sequence tiles
KT = E // 128     # 8 contraction tiles
MT = HD // 128    # 4 projection row tiles
LN_B = float(np.log(0.625))   # ln(TEMPERATURE * scaling) = ln(5/8)


def build_module():
    nc = bacc.Bacc(None, target_bir_lowering=False)

    xq = nc.dram_tensor("xq", [E, S], F32, kind="ExternalInput")   # query[b].T
    xk = nc.dram_tensor("xk", [E, S], F32, kind="ExternalInput")
    xv = nc.dram_tensor("xv", [E, S], F32, kind="ExternalInput")
    refT = nc.dram_tensor("refT", [3, S], F32, kind="ExternalInput")
    wq = nc.dram_tensor("wq", [E, HD], F32, kind="ExternalInput")  # Wq.T slice
    wk = nc.dram_tensor("wk", [E, HD], F32, kind="ExternalInput")
    wv = nc.dram_tensor("wv", [E, HD], F32, kind="ExternalInput")
    wo = nc.dram_tensor("wo", [HD, E], F32, kind="ExternalInput")  # Wo.T slice
    bqd = nc.dram_tensor("bq", [HD], F32, kind="ExternalInput")
    bkd = nc.dram_tensor("bk", [HD], F32, kind="ExternalInput")
    bvd = nc.dram_tensor("bv", [HD], F32, kind="ExternalInput")
    bod = nc.dram_tensor("bo", [E], F32, kind="ExternalInput")     # bo * 0.5
    out_d = nc.dram_tensor("out_part", [S, E], F32, kind="ExternalOutput")
    probs_d = nc.dram_tensor("probs_part", [HG, S, S], F32, kind="ExternalOutput")

    with tile.TileContext(nc) as tc:
        with (
            tc.tile_pool(name="const", bufs=1) as constp,
            tc.tile_pool(name="persist", bufs=1) as pers,
            tc.tile_pool(name="tmp", bufs=1) as tmp,
            tc.tile_pool(name="ub", bufs=1) as ubp,
            tc.tile_pool(name="pst", bufs=2) as pstp,
            tc.tile_pool(name="psA", bufs=2, space="PSUM") as psA,
            tc.tile_pool(name="psB", bufs=2, space="PSUM") as psB,
            tc.tile_pool(name="psU", bufs=1, space="PSUM") as psU,
        ):
            # ---- constants ----
            ident = constp.tile([128, 128], BF16, tag="ident")
            make_identity(nc, ident[:])
            ident_f = constp.tile([128, 128], F32, tag="ident_f")
            make_identity(nc, ident_f[:])
            ones_row = constp.tile([1, 128], BF16, tag="ones_row")
            nc.vector.memset(ones_row[:], 1.0)
            ones3 = constp.tile([3, 1], F32, tag="ones3")
            nc.vector.memset(ones3[:], 1.0)
            lnb = constp.tile([128, 1], F32, tag="lnb")
            nc.vector.memset(lnb[:], LN_B)
            bqs = constp.tile([128, MT], F32, tag="bqs")
            bks = constp.tile([128, MT], F32, tag="bks")
            with nc.allow_non_contiguous_dma("tiny bias loads"):
                nc.sync.dma_start(bqs[:], bqd.rearrange("(m p) -> p m", p=128))
                nc.sync.dma_start(bks[:], bkd.rearrange("(m p) -> p m", p=128))
            bvr = constp.tile([1, HD], BF16, tag="bvr")
            bor = constp.tile([1, E], BF16, tag="bor")

            # persistent activations / weights for later phases
            dwp = [pers.tile([128, S], F32, tag=f"dwp{i}", name=f"dwp{i}")
                   for i in range(NT)]
            qtb = [pers.tile([128, S], BF16, tag=f"qtb{m}", name=f"qtb{m}")
                   for m in range(MT)]
            ktb = [pers.tile([128, S], BF16, tag=f"ktb{m}", name=f"ktb{m}")
                   for m in range(MT)]
            vpb = [pers.tile([128, HG * 65], BF16, tag=f"vpb{i}", name=f"vpb{i}")
                   for i in range(NT)]
            otb = [pers.tile([128, S], BF16, tag=f"otb{m}", name=f"otb{m}")
                   for m in range(MT)]
            wob = [pers.tile([128, E], BF16, tag=f"wob{m}", name=f"wob{m}")
                   for m in range(MT)]

            # ================= phase 0/1: loads, dw, projections =============
            with (
                tc.tile_pool(name="p01", bufs=1) as p01,
                tc.tile_pool(name="stage", bufs=4) as stage,
            ):
                dma_engs = [nc.sync, nc.scalar, nc.gpsimd]
                dma_i = [0]

                def cast_load(dram, n_tiles, width, tag, eng, dst=None, dq=None):
                    tiles = []
                    for i in range(n_tiles):
                        st = stage.tile([128, 1024], F32, tag="stage", name=f"st_{tag}{i}")
                        q = dq if dq is not None else dma_engs[dma_i[0] % 3]
                        q.dma_start(st[:, :width], dram[ts(i, 128), :])
                        dma_i[0] += 1
                        if dst is None:
                            bt = p01.tile([128, width], BF16, tag=f"{tag}{i}",
                                          name=f"{tag}{i}")
                        else:
                            bt = dst[i]
                        e = eng if eng is not None else (nc.vector, nc.scalar)[i % 2]
                        if e is nc.scalar:
                            e.copy(bt[:], st[:, :width])
                        else:
                            e.tensor_copy(bt[:], st[:, :width])
                        tiles.append(bt)
                    return tiles

                # ---- distance weights dw' ----
                # 5-dim contraction rows: L = [-2r; n; 1], R = [r; 1; n]
                # (engine ops only start at partitions 0/32/64/96, so rows at
                # partitions 3 and 4 are placed with SBUF->SBUF DMA)
                reft = p01.tile([3, S], F32, tag="reft")
                nc.sync.dma_start(reft[:], refT[:])
                sq = p01.tile([3, S], F32, tag="sq")
                nc.vector.tensor_mul(sq[:], reft[:], reft[:])
                n_ps = psA.tile([1, S], F32, tag="psA", name="n_ps")
                for c in range(2):
                    nc.tensor.matmul(n_ps[:, ts(c, 512)], ones3[:], sq[:, ts(c, 512)],
                                     start=True, stop=True)
                n_sb = p01.tile([1, S], F32, tag="n_sb")
                nc.vector.tensor_copy(n_sb[:], n_ps[0:1, :])
                ones_f = p01.tile([1, S], F32, tag="ones_f")
                nc.vector.memset(ones_f[:], 1.0)
                R5 = p01.tile([5, S], F32, tag="R5")
                L5 = p01.tile([5, S], F32, tag="L5")
                nc.vector.tensor_copy(R5[0:3, :], reft[:])
                nc.sync.dma_start(R5[3:4, :], ones_f[:])
                nc.sync.dma_start(R5[4:5, :], n_sb[:])
                nc.vector.tensor_scalar_mul(L5[0:3, :], reft[:], -2.0)
                nc.sync.dma_start(L5[3:4, :], n_sb[:])
                nc.sync.dma_start(L5[4:5, :], ones_f[:])

                # all casts alternate DVE/ACT; GPSIMD's sequencer stays free
                # to act as a third input-DMA queue.
                xvb = cast_load(xv, KT, S, "xvb", None)
                wvb = cast_load(wv, KT, HD, "wvb", None)
                xqb = cast_load(xq, KT, S, "xqb", None)
                wqb = cast_load(wq, KT, HD, "wqb", None)
                xkb = cast_load(xk, KT, S, "xkb", None)
                wkb = cast_load(wk, KT, HD, "wkb", None)
                cast_load(wo, MT, E, "wob", nc.gpsimd, dst=wob)

                # small bias rows
                bvr_f = stage.tile([1, HD], F32, tag="stage", name="bvr_f")
                nc.sync.dma_start(bvr_f[:], bvd.rearrange("(a n) -> a n", a=1))
                nc.vector.tensor_copy(bvr[:], bvr_f[:])
                bor_f = stage.tile([1, E], F32, tag="stage", name="bor_f")
                nc.sync.dma_start(bor_f[:], bod.rearrange("(a n) -> a n", a=1))
                nc.vector.tensor_copy(bor[:], bor_f[:])

                for sp in range(NT):
                    d_ps = psA.tile([128, S], F32, tag="psA", name=f"d_ps{sp}")
                    for c in range(2):
                        nc.tensor.matmul(d_ps[:, ts(c, 512)], L5[:, ts(sp, 128)],
                                         R5[:, ts(c, 512)], start=True, stop=True)
                    nc.vector.tensor_scalar(out=dwp[sp][:], in0=d_ps[:], scalar1=1.0,
                                            scalar2=900.0, op0=ALU.max, op1=ALU.min)
                for sp in range(NT):
                    v_ps = psA.tile([128, S], F32, tag="psA", name=f"v_ps{sp}")
                    for k in range(KT):
                        nc.tensor.matmul(v_ps[:, 0:HD], xvb[k][:, ts(sp, 128)],
                                         wvb[k][:], start=(k == 0), stop=False)
                    nc.tensor.matmul(v_ps[:, 0:HD], ones_row[:], bvr[:],
                                     start=False, stop=True)
                    v3 = vpb[sp].rearrange("p (h c) -> p h c", c=65)
                    nc.vector.tensor_copy(
                        v3[:, :, 0:64],
                        v_ps[:, 0:HD].rearrange("p (h d) -> p h d", d=64))
                    nc.vector.memset(v3[:, :, 64:65], 1.0)

                # ---- QKV projections ----
                for m in range(MT):
                    for which in range(2):
                        wb = (wqb, wkb)[which]
                        xb = (xqb, xkb)[which]
                        bias_col = (bqs, bks)[which]
                        dst = (qtb, ktb)[which]
                        p_ps = psA.tile([128, S], F32, tag="psA",
                                        name=f"p_ps{which}{m}")
                        for c in range(2):
                            for k in range(KT):
                                nc.tensor.matmul(p_ps[:, ts(c, 512)],
                                                 wb[k][:, ts(m, 128)],
                                                 xb[k][:, ts(c, 512)],
                                                 start=(k == 0), stop=(k == KT - 1))
                        nc.vector.tensor_scalar(out=dst[m][:], in0=p_ps[:],
                                                scalar1=bias_col[:, m:m + 1],
                                                scalar2=None, op0=ALU.add)


                # batched by table set: all Ln, then all Exp (2 loads)
                for sp in range(NT):
                    nc.scalar.activation(dwp[sp][:], dwp[sp][:], AF.Ln)
                for sp in range(NT):
                    nc.scalar.activation(dwp[sp][:], dwp[sp][:], AF.Exp,
                                         scale=-0.5, bias=lnb[:])

            # ================= phase 2: per-head attention ===================
            with tc.tile_pool(name="et", bufs=1) as etp:
                for h in range(HG):
                    km = h // 2
                    ro = (h % 2) * 64
                    # et_big[j] holds E^T for sp = 2j, 2j+1 -> one exp per 2 sp
                    et_big = [etp.tile([128, 2 * S], BF16, tag=f"et{j}", bufs=2,
                                       name=f"et{h}_{j}") for j in range(NT // 2)]

                    def et_sl(sp, col, width):
                        return et_big[sp // 2][:, (sp % 2) * S + col:
                                               (sp % 2) * S + col + width]

                    u_ps = psU.tile([65, S], F32, tag="psU", name=f"u_ps{h}")
                    for sp in range(NT):
                        s_ps = psA.tile([128, S], F32, tag="psA",
                                        name=f"s_ps{h}_{sp}")
                        for c in range(2):
                            nc.tensor.matmul(s_ps[:, ts(c, 512)],
                                             ktb[km][ro:ro + 64, ts(sp, 128)],
                                             qtb[km][ro:ro + 64, ts(c, 512)],
                                             start=True, stop=True)
                        # tmod in place in PSUM; exp reads PSUM directly
                        nc.vector.tensor_tensor(out=s_ps[:], in0=s_ps[:],
                                                in1=dwp[sp][:], op=ALU.mult)
                        nc.scalar.activation(et_sl(sp, 0, S), s_ps[:], AF.Exp)
                        # PV accumulation interleaved per sp
                        for c in range(2):
                            nc.tensor.matmul(u_ps[:, ts(c, 512)],
                                             vpb[sp][:, h * 65:(h + 1) * 65],
                                             et_sl(sp, c * 512, 512),
                                             start=(sp == 0), stop=(sp == NT - 1))
                    ub = ubp.tile([65, S], F32, tag="ub", name=f"ub{h}")
                    nc.vector.tensor_copy(ub[:], u_ps[:])

                    for tt in range(NT):
                        up_ps = psB.tile([128, 65], F32, tag="psB", name=f"up{h}_{tt}")
                        nc.tensor.transpose(up_ps[:], ub[:, ts(tt, 128)],
                                            ident_f[0:65, 0:65])
                        rz = tmp.tile([128, 1], F32, tag="rz", bufs=4,
                                      name=f"rz{h}_{tt}")
                        nc.vector.reciprocal(rz[:], up_ps[:, 64:65])
                        o_sb = tmp.tile([128, 64], BF16, tag="osb", bufs=4,
                                        name=f"osb{h}_{tt}")
                        nc.scalar.activation(o_sb[:], up_ps[:, 0:64], AF.Copy,
                                             scale=rz[:])
                        ot_ps = psB.tile([64, 128], BF16, tag="psB",
                                         name=f"otp{h}_{tt}")
                        nc.tensor.transpose(ot_ps[:], o_sb[:], ident[:])
                        nc.scalar.copy(otb[km][ro:ro + 64, ts(tt, 128)], ot_ps[:])

                        p_ps = psB.tile([128, S], BF16, tag="psB",
                                        name=f"pp{h}_{tt}")
                        for sp in range(NT):
                            nc.tensor.transpose(p_ps[:, ts(sp, 128)],
                                                et_sl(sp, tt * 128, 128), ident[:])
                        pt = pstp.tile([128, S], F32, tag="pst", name=f"pt{h}_{tt}")
                        if (h * NT + tt) % 2 == 0:
                            nc.vector.tensor_scalar(out=pt[:], in0=p_ps[:],
                                                    scalar1=rz[:], scalar2=None,
                                                    op0=ALU.mult)
                        else:
                            nc.scalar.activation(pt[:], p_ps[:], AF.Copy,
                                                 scale=rz[:])
                        out_eng = (nc.sync, nc.gpsimd)[(h * NT + tt) % 2]
                        out_eng.dma_start(probs_d[h, ts(tt, 128), :], pt[:])

                # ---- output projection ----
                for tt in range(NT):
                    o_ps = psA.tile([128, S], F32, tag="psA", name=f"o_ps{tt}")
                    for c in range(2):
                        for m in range(MT):
                            nc.tensor.matmul(o_ps[:, ts(c, 512)],
                                             otb[m][:, ts(tt, 128)],
                                             wob[m][:, ts(c, 512)],
                                             start=(m == 0), stop=False)
                        nc.tensor.matmul(o_ps[:, ts(c, 512)], ones_row[:],
                                         bor[:, ts(c, 512)], start=False, stop=True)
                    osb = pstp.tile([128, S], F32, tag="pst", name=f"ov{tt}")
                    if tt % 2 == 0:
                        nc.scalar.copy(osb[:], o_ps[:])
                    else:
                        nc.vector.tensor_copy(osb[:], o_ps[:])
                    nc.sync.dma_start(out_d[ts(tt, 128), :], osb[:])

    nc.compile()
    return nc


_NC = None


def _get_nc():
    global _NC
    if _NC is None:
        _NC = build_module()
    return _NC


def shard_inputs(query, key, value, coords, Wq, bq, Wk, bk, Wv, bv, Wo, bo):
    """Build the 8 per-core input maps (pure slicing / layout, no math)."""
    in_maps = []
    WqT, WkT, WvT, WoT = Wq.T, Wk.T, Wv.T, Wo.T
    bo_half = (bo * 0.5).astype(np.float32)
    for c in range(8):
        b, hg = c // 2, c % 2
        sl = slice(hg * HD, (hg + 1) * HD)
        in_maps.append({
            "xq": np.ascontiguousarray(query[b].T),
            "xk": np.ascontiguousarray(key[b].T),
            "xv": np.ascontiguousarray(value[b].T),
            "refT": np.ascontiguousarray(coords[b, :, 2, :].T),
            "wq": np.ascontiguousarray(WqT[:, sl]),
            "wk": np.ascontiguousarray(WkT[:, sl]),
            "wv": np.ascontiguousarray(WvT[:, sl]),
            "wo": np.ascontiguousarray(WoT[sl, :]),
            "bq": np.ascontiguousarray(bq[sl]),
            "bk": np.ascontiguousarray(bk[sl]),
            "bv": np.ascontiguousarray(bv[sl]),
            "bo": bo_half,
        })
    return in_maps


def kernel(query, key, value, coords, Wq, bq, Wk, bk, Wv, bv, Wo, bo, _trace=False):
    args = [np.asarray(a, np.float32) for a in
            (query, key, value, coords, Wq, bq, Wk, bk, Wv, bv, Wo, bo)]
    nc = _get_nc()
    in_maps = shard_inputs(*args)
    res = run_bass_kernel_spmd(nc, in_maps, core_ids=list(range(8)), trace=_trace)
    B = query.shape[0]
    out = np.zeros((B, S, E), np.float32)
    probs = np.zeros((B, 2 * HG, S, S), np.float32)
    for c in range(8):
        b, hg = c // 2, c % 2
        out[b] += res.results[c]["out_part"]
        probs[b, hg * HG:(hg + 1) * HG] = res.results[c]["probs_part"]
    kernel.last_exec_time_ns = res.exec_time_ns
    kernel.last_results = res
    return out, probs


# revision 27
# speedup vs baseline: 1.0753x; 1.0753x over previous
"""Distance-modulated attention on 8 Trainium2 NeuronCores (Bass/Tile).

Sharding: core c handles batch b = c//2 and head-group hg = c%2 (8 of 16 heads).
Tensor-parallel over heads for QKV/out projections; per-batch distance matrix
is computed on-device per core. Host only slices/transposes (layout), concats,
and sums the two head-group partial outputs per batch (the TP all-reduce).

Per-core device pipeline (S=1024, E=1024, 8 heads x D=64):
  - cast inputs/weights fp32->bf16 on DVE/ACT/GPSIMD
  - d2 via K=5 fp32 matmul of [n_i,1,-2r] x [1,n_j,r]; dw' = 0.625/clip(sqrt(d2),1,30)
    computed as exp(-0.5*ln(clip(d2,1,900)) + ln(0.625)) on ACT (one table set)
  - QT = Wq'^T X^T, KT likewise (transposed layout), V = X Wv' (normal layout,
    bias via K=1 ones-row matmul), all bf16 MMs with fp32 PSUM accumulation
  - per head: S^T = K Q^T (K=64 bf16 MM) -> tmod = S^T * dw' (DVE fp32)
    -> E^T = exp(tmod) (ACT, bf16 out) -> U'^T = [V|1]^T E^T (PV matmul;
    row 64 = softmax denominator Z) -> per 128-row tile: PE-transpose U',
    rz = 1/Z (DVE reciprocal), O = U/Z (bf16), PE-transpose back to O^T;
    probs: PE-transpose E^T tiles, evict*rz to fp32 (DVE/ACT alternating), DMA
  - out = O^T.T Wo' + bo/2 (bf16 MM, bias via ones-row matmul)
"""
import numpy as np
import concourse.bass as bass
import concourse.tile as tile
import concourse.mybir as mybir
from concourse import bacc
from concourse.bass import ts
from concourse.bass_utils import run_bass_kernel_spmd
from concourse.masks import make_identity

F32 = mybir.dt.float32
BF16 = mybir.dt.bfloat16
AF = mybir.ActivationFunctionType
ALU = mybir.AluOpType

S = 1024          # sequence length
E = 1024          # embed dim
HG = 8            # heads per core
D = 64            # head dim
HD = HG * D       # 512, per-core projection width
NT = S // 128     # 8 sequence tiles
KT = E // 128     # 8 contraction tiles
MT = HD // 128    # 4 projection row tiles
LN_B = float(np.log(0.625))   # ln(TEMPERATURE * scaling) = ln(5/8)


def build_module():
    nc = bacc.Bacc(None, target_bir_lowering=False)

    xq = nc.dram_tensor("xq", [E, S], F32, kind="ExternalInput")   # query[b].T
    xk = nc.dram_tensor("xk", [E, S], F32, kind="ExternalInput")
    xv = nc.dram_tensor("xv", [E, S], F32, kind="ExternalInput")
    refT = nc.dram_tensor("refT", [3, S], F32, kind="ExternalInput")
    wq = nc.dram_tensor("wq", [E, HD], F32, kind="ExternalInput")  # Wq.T slice
    wk = nc.dram_tensor("wk", [E, HD], F32, kind="ExternalInput")
    wv = nc.dram_tensor("wv", [E, HD], F32, kind="ExternalInput")
    wo = nc.dram_tensor("wo", [HD, E], F32, kind="ExternalInput")  # Wo.T slice
    bqd = nc.dram_tensor("bq", [HD], F32, kind="ExternalInput")
    bkd = nc.dram_tensor("bk", [HD], F32, kind="ExternalInput")
    bvd = nc.dram_tensor("bv", [HD], F32, kind="ExternalInput")
    bod = nc.dram_tensor("bo", [E], F32, kind="ExternalInput")     # bo * 0.5
    out_d = nc.dram_tensor("out_part", [S, E], F32, kind="ExternalOutput")
    probs_d = nc.dram_tensor("probs_part", [HG, S, S], F32, kind="ExternalOutput")

    with tile.TileContext(nc) as tc:
        with (
            tc.tile_pool(name="const", bufs=1) as constp,
            tc.tile_pool(name="persist", bufs=1) as pers,
            tc.tile_pool(name="tmp", bufs=1) as tmp,
            tc.tile_pool(name="ub", bufs=1) as ubp,
            tc.tile_pool(name="pst", bufs=2) as pstp,
            tc.tile_pool(name="psA", bufs=2, space="PSUM") as psA,
            tc.tile_pool(name="psB", bufs=2, space="PSUM") as psB,
            tc.tile_pool(name="psU", bufs=1, space="PSUM") as psU,
        ):
            # ---- constants ----
            ident = constp.tile([128, 128], BF16, tag="ident")
            make_identity(nc, ident[:])
            ident_f = constp.tile([128, 128], F32, tag="ident_f")
            make_identity(nc, ident_f[:])
            ones_row = constp.tile([1, 128], BF16, tag="ones_row")
            nc.vector.memset(ones_row[:], 1.0)
            ones3 = constp.tile([3, 1], F32, tag="ones3")
            nc.vector.memset(ones3[:], 1.0)
            lnb = constp.tile([128, 1], F32, tag="lnb")
            nc.vector.memset(lnb[:], LN_B)
            bqs = constp.tile([128, MT], F32, tag="bqs")
            bks = constp.tile([128, MT], F32, tag="bks")
            with nc.allow_non_contiguous_dma("tiny bias loads"):
                nc.sync.dma_start(bqs[:], bqd.rearrange("(m p) -> p m", p=128))
                nc.sync.dma_start(bks[:], bkd.rearrange("(m p) -> p m", p=128))
            bvr = constp.tile([1, HD], BF16, tag="bvr")
            bor = constp.tile([1, E], BF16, tag="bor")

            # persistent activations / weights for later phases
            dwp = [pers.tile([128, S], F32, tag=f"dwp{i}", name=f"dwp{i}")
                   for i in range(NT)]
            qtb = [pers.tile([128, S], BF16, tag=f"qtb{m}", name=f"qtb{m}")
                   for m in range(MT)]
            ktb = [pers.tile([128, S], BF16, tag=f"ktb{m}", name=f"ktb{m}")
                   for m in range(MT)]
            vpb = [pers.tile([128, HG * 65], BF16, tag=f"vpb{i}", name=f"vpb{i}")
                   for i in range(NT)]
            otb = [pers.tile([128, S], BF16, tag=f"otb{m}", name=f"otb{m}")
                   for m in range(MT)]
            wob = [pers.tile([128, E], BF16, tag=f"wob{m}", name=f"wob{m}")
                   for m in range(MT)]

            # ================= phase 0/1: loads, dw, projections =============
            with (
                tc.tile_pool(name="p01", bufs=1) as p01,
                tc.tile_pool(name="stage", bufs=4) as stage,
            ):
                dma_engs = [nc.sync, nc.scalar, nc.gpsimd]
                dma_i = [0]

                def cast_load(dram, n_tiles, width, tag, eng, dst=None, dq=None):
                    tiles = []
                    for i in range(n_tiles):
                        st = stage.tile([128, 1024], F32, tag="stage", name=f"st_{tag}{i}")
                        q = dq if dq is not None else dma_engs[dma_i[0] % 3]
                        q.dma_start(st[:, :width], dram[ts(i, 128), :])
                        dma_i[0] += 1
                        if dst is None:
                            bt = p01.tile([128, width], BF16, tag=f"{tag}{i}",
                                          name=f"{tag}{i}")
                        else:
                            bt = dst[i]
                        e = eng if eng is not None else (nc.vector, nc.scalar)[i % 2]
                        if e is nc.scalar:
                            e.copy(bt[:], st[:, :width])
                        else:
                            e.tensor_copy(bt[:], st[:, :width])
                        tiles.append(bt)
                    return tiles

                # ---- distance weights dw' ----
                # 5-dim contraction rows: L = [-2r; n; 1], R = [r; 1; n]
                # (engine ops only start at partitions 0/32/64/96, so rows at
                # partitions 3 and 4 are placed with SBUF->SBUF DMA)
                reft = p01.tile([3, S], F32, tag="reft")
                nc.sync.dma_start(reft[:], refT[:])
                sq = p01.tile([3, S], F32, tag="sq")
                nc.vector.tensor_mul(sq[:], reft[:], reft[:])
                n_ps = psA.tile([1, S], F32, tag="psA", name="n_ps")
                for c in range(2):
                    nc.tensor.matmul(n_ps[:, ts(c, 512)], ones3[:], sq[:, ts(c, 512)],
                                     start=True, stop=True)
                n_sb = p01.tile([1, S], F32, tag="n_sb")
                nc.vector.tensor_copy(n_sb[:], n_ps[0:1, :])
                ones_f = p01.tile([1, S], F32, tag="ones_f")
                nc.vector.memset(ones_f[:], 1.0)
                R5 = p01.tile([5, S], F32, tag="R5")
                L5 = p01.tile([5, S], F32, tag="L5")
                nc.vector.tensor_copy(R5[0:3, :], reft[:])
                nc.sync.dma_start(R5[3:4, :], ones_f[:])
                nc.sync.dma_start(R5[4:5, :], n_sb[:])
                nc.vector.tensor_scalar_mul(L5[0:3, :], reft[:], -2.0)
                nc.sync.dma_start(L5[3:4, :], n_sb[:])
                nc.sync.dma_start(L5[4:5, :], ones_f[:])

                # all casts alternate DVE/ACT; GPSIMD's sequencer stays free
                # to act as a third input-DMA queue.
                xqb = cast_load(xq, KT, S, "xqb", None)
                wqb = cast_load(wq, KT, HD, "wqb", None)
                xkb = cast_load(xk, KT, S, "xkb", None)
                wkb = cast_load(wk, KT, HD, "wkb", None)
                xvb = cast_load(xv, KT, S, "xvb", None)
                wvb = cast_load(wv, KT, HD, "wvb", None)
                cast_load(wo, MT, E, "wob", nc.gpsimd, dst=wob)

                # small bias rows
                bvr_f = stage.tile([1, HD], F32, tag="stage", name="bvr_f")
                nc.sync.dma_start(bvr_f[:], bvd.rearrange("(a n) -> a n", a=1))
                nc.vector.tensor_copy(bvr[:], bvr_f[:])
                bor_f = stage.tile([1, E], F32, tag="stage", name="bor_f")
                nc.sync.dma_start(bor_f[:], bod.rearrange("(a n) -> a n", a=1))
                nc.vector.tensor_copy(bor[:], bor_f[:])

                for sp in range(NT):
                    d_ps = psA.tile([128, S], F32, tag="psA", name=f"d_ps{sp}")
                    for c in range(2):
                        nc.tensor.matmul(d_ps[:, ts(c, 512)], L5[:, ts(sp, 128)],
                                         R5[:, ts(c, 512)], start=True, stop=True)
                    nc.vector.tensor_scalar(out=dwp[sp][:], in0=d_ps[:], scalar1=1.0,
                                            scalar2=900.0, op0=ALU.max, op1=ALU.min)
                # ---- QKV projections ----
                for m in range(MT):
                    for which in range(2):
                        wb = (wqb, wkb)[which]
                        xb = (xqb, xkb)[which]
                        bias_col = (bqs, bks)[which]
                        dst = (qtb, ktb)[which]
                        p_ps = psA.tile([128, S], F32, tag="psA",
                                        name=f"p_ps{which}{m}")
                        for c in range(2):
                            for k in range(KT):
                                nc.tensor.matmul(p_ps[:, ts(c, 512)],
                                                 wb[k][:, ts(m, 128)],
                                                 xb[k][:, ts(c, 512)],
                                                 start=(k == 0), stop=(k == KT - 1))
                        nc.vector.tensor_scalar(out=dst[m][:], in0=p_ps[:],
                                                scalar1=bias_col[:, m:m + 1],
                                                scalar2=None, op0=ALU.add)

                for sp in range(NT):
                    v_ps = psA.tile([128, S], F32, tag="psA", name=f"v_ps{sp}")
                    for k in range(KT):
                        nc.tensor.matmul(v_ps[:, 0:HD], xvb[k][:, ts(sp, 128)],
                                         wvb[k][:], start=(k == 0), stop=False)
                    nc.tensor.matmul(v_ps[:, 0:HD], ones_row[:], bvr[:],
                                     start=False, stop=True)
                    v3 = vpb[sp].rearrange("p (h c) -> p h c", c=65)
                    nc.vector.tensor_copy(
                        v3[:, :, 0:64],
                        v_ps[:, 0:HD].rearrange("p (h d) -> p h d", d=64))
                    nc.vector.memset(v3[:, :, 64:65], 1.0)



                # batched by table set: all Ln, then all Exp (2 loads)
                for sp in range(NT):
                    nc.scalar.activation(dwp[sp][:], dwp[sp][:], AF.Ln)
                for sp in range(NT):
                    nc.scalar.activation(dwp[sp][:], dwp[sp][:], AF.Exp,
                                         scale=-0.5, bias=lnb[:])

            # ================= phase 2: per-head attention ===================
            with tc.tile_pool(name="et", bufs=1) as etp:
                for h in range(HG):
                    km = h // 2
                    ro = (h % 2) * 64
                    # et_big[j] holds E^T for sp = 2j, 2j+1 -> one exp per 2 sp
                    et_big = [etp.tile([128, 2 * S], BF16, tag=f"et{j}", bufs=2,
                                       name=f"et{h}_{j}") for j in range(NT // 2)]

                    def et_sl(sp, col, width):
                        return et_big[sp // 2][:, (sp % 2) * S + col:
                                               (sp % 2) * S + col + width]

                    u_ps = psU.tile([65, S], F32, tag="psU", name=f"u_ps{h}")
                    for sp in range(NT):
                        s_ps = psA.tile([128, S], F32, tag="psA",
                                        name=f"s_ps{h}_{sp}")
                        for c in range(2):
                            nc.tensor.matmul(s_ps[:, ts(c, 512)],
                                             ktb[km][ro:ro + 64, ts(sp, 128)],
                                             qtb[km][ro:ro + 64, ts(c, 512)],
                                             start=True, stop=True)
                        # tmod in place in PSUM; exp reads PSUM directly
                        nc.vector.tensor_tensor(out=s_ps[:], in0=s_ps[:],
                                                in1=dwp[sp][:], op=ALU.mult)
                        nc.scalar.activation(et_sl(sp, 0, S), s_ps[:], AF.Exp)
                        # PV accumulation interleaved per sp
                        for c in range(2):
                            nc.tensor.matmul(u_ps[:, ts(c, 512)],
                                             vpb[sp][:, h * 65:(h + 1) * 65],
                                             et_sl(sp, c * 512, 512),
                                             start=(sp == 0), stop=(sp == NT - 1))
                    ub = ubp.tile([65, S], F32, tag="ub", name=f"ub{h}")
                    nc.vector.tensor_copy(ub[:], u_ps[:])

                    for tt in range(NT):
                        up_ps = psB.tile([128, 65], F32, tag="psB", name=f"up{h}_{tt}")
                        nc.tensor.transpose(up_ps[:], ub[:, ts(tt, 128)],
                                            ident_f[0:65, 0:65])
                        rz = tmp.tile([128, 1], F32, tag="rz", bufs=4,
                                      name=f"rz{h}_{tt}")
                        nc.vector.reciprocal(rz[:], up_ps[:, 64:65])
                        o_sb = tmp.tile([128, 64], BF16, tag="osb", bufs=4,
                                        name=f"osb{h}_{tt}")
                        nc.scalar.activation(o_sb[:], up_ps[:, 0:64], AF.Copy,
                                             scale=rz[:])
                        ot_ps = psB.tile([64, 128], BF16, tag="psB",
                                         name=f"otp{h}_{tt}")
                        nc.tensor.transpose(ot_ps[:], o_sb[:], ident[:])
                        nc.scalar.copy(otb[km][ro:ro + 64, ts(tt, 128)], ot_ps[:])

                        p_ps = psB.tile([128, S], BF16, tag="psB",
                                        name=f"pp{h}_{tt}")
                        for sp in range(NT):
                            nc.tensor.transpose(p_ps[:, ts(sp, 128)],
                                                et_sl(sp, tt * 128, 128), ident[:])
                        pt = pstp.tile([128, S], F32, tag="pst", name=f"pt{h}_{tt}")
                        if (h * NT + tt) % 2 == 0:
                            nc.vector.tensor_scalar(out=pt[:], in0=p_ps[:],
                                                    scalar1=rz[:], scalar2=None,
                                                    op0=ALU.mult)
                        else:
                            nc.scalar.activation(pt[:], p_ps[:], AF.Copy,
                                                 scale=rz[:])
                        out_eng = (nc.sync, nc.gpsimd)[(h * NT + tt) % 2]
                        out_eng.dma_start(probs_d[h, ts(tt, 128), :], pt[:])

                # ---- output projection ----
                for tt in range(NT):
                    o_ps = psA.tile([128, S], F32, tag="psA", name=f"o_ps{tt}")
                    for c in range(2):
                        for m in range(MT):
                            nc.tensor.matmul(o_ps[:, ts(c, 512)],
                                             otb[m][:, ts(tt, 128)],
                                             wob[m][:, ts(c, 512)],
                                             start=(m == 0), stop=False)
                        nc.tensor.matmul(o_ps[:, ts(c, 512)], ones_row[:],
                                         bor[:, ts(c, 512)], start=False, stop=True)
                    osb = pstp.tile([128, S], F32, tag="pst", name=f"ov{tt}")
                    if tt % 2 == 0:
                        nc.scalar.copy(osb[:], o_ps[:])
                    else:
                        nc.vector.tensor_copy(osb[:], o_ps[:])
                    nc.sync.dma_start(out_d[ts(tt, 128), :], osb[:])

    nc.compile()
    return nc


_NC = None


def _get_nc():
    global _NC
    if _NC is None:
        _NC = build_module()
    return _NC


def shard_inputs(query, key, value, coords, Wq, bq, Wk, bk, Wv, bv, Wo, bo):
    """Build the 8 per-core input maps (pure slicing / layout, no math)."""
    in_maps = []
    WqT, WkT, WvT, WoT = Wq.T, Wk.T, Wv.T, Wo.T
    bo_half = (bo * 0.5).astype(np.float32)
    for c in range(8):
        b, hg = c // 2, c % 2
        sl = slice(hg * HD, (hg + 1) * HD)
        in_maps.append({
            "xq": np.ascontiguousarray(query[b].T),
            "xk": np.ascontiguousarray(key[b].T),
            "xv": np.ascontiguousarray(value[b].T),
            "refT": np.ascontiguousarray(coords[b, :, 2, :].T),
            "wq": np.ascontiguousarray(WqT[:, sl]),
            "wk": np.ascontiguousarray(WkT[:, sl]),
            "wv": np.ascontiguousarray(WvT[:, sl]),
            "wo": np.ascontiguousarray(WoT[sl, :]),
            "bq": np.ascontiguousarray(bq[sl]),
            "bk": np.ascontiguousarray(bk[sl]),
            "bv": np.ascontiguousarray(bv[sl]),
            "bo": bo_half,
        })
    return in_maps


def kernel(query, key, value, coords, Wq, bq, Wk, bk, Wv, bv, Wo, bo, _trace=False):
    args = [np.asarray(a, np.float32) for a in
            (query, key, value, coords, Wq, bq, Wk, bk, Wv, bv, Wo, bo)]
    nc = _get_nc()
    in_maps = shard_inputs(*args)
    res = run_bass_kernel_spmd(nc, in_maps, core_ids=list(range(8)), trace=_trace)
    B = query.shape[0]
    out = np.zeros((B, S, E), np.float32)
    probs = np.zeros((B, 2 * HG, S, S), np.float32)
    for c in range(8):
        b, hg = c // 2, c % 2
        out[b] += res.results[c]["out_part"]
        probs[b, hg * HG:(hg + 1) * HG] = res.results[c]["probs_part"]
    kernel.last_exec_time_ns = res.exec_time_ns
    kernel.last_results = res
    return out, probs


# revision 28
# speedup vs baseline: 1.0820x; 1.0063x over previous
"""Distance-modulated attention on 8 Trainium2 NeuronCores (Bass/Tile).

Sharding: core c handles batch b = c//2 and head-group hg = c%2 (8 of 16 heads).
Tensor-parallel over heads for QKV/out projections; per-batch distance matrix
is computed on-device per core. Host only slices/transposes (layout), concats,
and sums the two head-group partial outputs per batch (the TP all-reduce).

Per-core device pipeline (S=1024, E=1024, 8 heads x D=64):
  - cast inputs/weights fp32->bf16 on DVE/ACT/GPSIMD
  - d2 via K=5 fp32 matmul of [n_i,1,-2r] x [1,n_j,r]; dw' = 0.625/clip(sqrt(d2),1,30)
    computed as exp(-0.5*ln(clip(d2,1,900)) + ln(0.625)) on ACT (one table set)
  - QT = Wq'^T X^T, KT likewise (transposed layout), V = X Wv' (normal layout,
    bias via K=1 ones-row matmul), all bf16 MMs with fp32 PSUM accumulation
  - per head: S^T = K Q^T (K=64 bf16 MM) -> tmod = S^T * dw' (DVE fp32)
    -> E^T = exp(tmod) (ACT, bf16 out) -> U'^T = [V|1]^T E^T (PV matmul;
    row 64 = softmax denominator Z) -> per 128-row tile: PE-transpose U',
    rz = 1/Z (DVE reciprocal), O = U/Z (bf16), PE-transpose back to O^T;
    probs: PE-transpose E^T tiles, evict*rz to fp32 (DVE/ACT alternating), DMA
  - out = O^T.T Wo' + bo/2 (bf16 MM, bias via ones-row matmul)
"""
import numpy as np
import concourse.bass as bass
import concourse.tile as tile
import concourse.mybir as mybir
from concourse import bacc
from concourse.bass import ts
from concourse.bass_utils import run_bass_kernel_spmd
from concourse.masks import make_identity

F32 = mybir.dt.float32
BF16 = mybir.dt.bfloat16
AF = mybir.ActivationFunctionType
ALU = mybir.AluOpType

S = 1024          # sequence length
E = 1024          # embed dim
HG = 8            # heads per core
D = 64            # head dim
HD = HG * D       # 512, per-core projection width
NT = S // 128     # 8 sequence tiles
KT = E // 128     # 8 contraction tiles
MT = HD // 128    # 4 projection row tiles
LN_B = float(np.log(0.625))   # ln(TEMPERATURE * scaling) = ln(5/8)


def build_module():
    nc = bacc.Bacc(None, target_bir_lowering=False)

    xq = nc.dram_tensor("xq", [E, S], F32, kind="ExternalInput")   # query[b].T
    xk = nc.dram_tensor("xk", [E, S], F32, kind="ExternalInput")
    xv = nc.dram_tensor("xv", [E, S], F32, kind="ExternalInput")
    refT = nc.dram_tensor("refT", [3, S], F32, kind="ExternalInput")
    wq = nc.dram_tensor("wq", [E, HD], F32, kind="ExternalInput")  # Wq.T slice
    wk = nc.dram_tensor("wk", [E, HD], F32, kind="ExternalInput")
    wv = nc.dram_tensor("wv", [E, HD], F32, kind="ExternalInput")
    wo = nc.dram_tensor("wo", [HD, E], F32, kind="ExternalInput")  # Wo.T slice
    bqd = nc.dram_tensor("bq", [HD], F32, kind="ExternalInput")
    bkd = nc.dram_tensor("bk", [HD], F32, kind="ExternalInput")
    bvd = nc.dram_tensor("bv", [HD], F32, kind="ExternalInput")
    bod = nc.dram_tensor("bo", [E], F32, kind="ExternalInput")     # bo * 0.5
    out_d = nc.dram_tensor("out_part", [S, E], F32, kind="ExternalOutput")
    probs_d = nc.dram_tensor("probs_part", [HG, S, S], F32, kind="ExternalOutput")

    with tile.TileContext(nc) as tc:
        with (
            tc.tile_pool(name="const", bufs=1) as constp,
            tc.tile_pool(name="persist", bufs=1) as pers,
            tc.tile_pool(name="tmp", bufs=1) as tmp,
            tc.tile_pool(name="ub", bufs=1) as ubp,
            tc.tile_pool(name="pst", bufs=2) as pstp,
            tc.tile_pool(name="psA", bufs=2, space="PSUM") as psA,
            tc.tile_pool(name="psB", bufs=2, space="PSUM") as psB,
            tc.tile_pool(name="psU", bufs=1, space="PSUM") as psU,
        ):
            # ---- constants ----
            ident = constp.tile([128, 128], BF16, tag="ident")
            make_identity(nc, ident[:])
            ident_f = constp.tile([128, 128], F32, tag="ident_f")
            make_identity(nc, ident_f[:])
            ones_row = constp.tile([1, 128], BF16, tag="ones_row")
            nc.vector.memset(ones_row[:], 1.0)
            ones3 = constp.tile([3, 1], F32, tag="ones3")
            nc.vector.memset(ones3[:], 1.0)
            lnb = constp.tile([128, 1], F32, tag="lnb")
            nc.vector.memset(lnb[:], LN_B)
            bqs = constp.tile([128, MT], F32, tag="bqs")
            bks = constp.tile([128, MT], F32, tag="bks")
            with nc.allow_non_contiguous_dma("tiny bias loads"):
                nc.sync.dma_start(bqs[:], bqd.rearrange("(m p) -> p m", p=128))
                nc.sync.dma_start(bks[:], bkd.rearrange("(m p) -> p m", p=128))
            bvr = constp.tile([1, HD], BF16, tag="bvr")
            bor = constp.tile([1, E], BF16, tag="bor")

            # persistent activations / weights for later phases
            dwp = [pers.tile([128, S], F32, tag=f"dwp{i}", name=f"dwp{i}")
                   for i in range(NT)]
            qtb = [pers.tile([128, S], BF16, tag=f"qtb{m}", name=f"qtb{m}")
                   for m in range(MT)]
            ktb = [pers.tile([128, S], BF16, tag=f"ktb{m}", name=f"ktb{m}")
                   for m in range(MT)]
            vpb = [pers.tile([128, HG * 65], BF16, tag=f"vpb{i}", name=f"vpb{i}")
                   for i in range(NT)]
            otb = [pers.tile([128, S], BF16, tag=f"otb{m}", name=f"otb{m}")
                   for m in range(MT)]
            wob = [pers.tile([128, E], BF16, tag=f"wob{m}", name=f"wob{m}")
                   for m in range(MT)]

            # ================= phase 0/1: loads, dw, projections =============
            with (
                tc.tile_pool(name="p01", bufs=1) as p01,
                tc.tile_pool(name="stage", bufs=4) as stage,
            ):
                dma_engs = [nc.sync, nc.scalar, nc.gpsimd]
                dma_i = [0]

                def cast_load(dram, n_tiles, width, tag, eng, dst=None, dq=None):
                    tiles = []
                    for i in range(n_tiles):
                        st = stage.tile([128, 1024], F32, tag="stage", name=f"st_{tag}{i}")
                        q = dq if dq is not None else dma_engs[dma_i[0] % 3]
                        q.dma_start(st[:, :width], dram[ts(i, 128), :])
                        dma_i[0] += 1
                        if dst is None:
                            bt = p01.tile([128, width], BF16, tag=f"{tag}{i}",
                                          name=f"{tag}{i}")
                        else:
                            bt = dst[i]
                        e = eng if eng is not None else (nc.vector, nc.scalar)[i % 2]
                        if e is nc.scalar:
                            e.copy(bt[:], st[:, :width])
                        else:
                            e.tensor_copy(bt[:], st[:, :width])
                        tiles.append(bt)
                    return tiles

                # ---- distance weights dw' ----
                # 5-dim contraction rows: L = [-2r; n; 1], R = [r; 1; n]
                # (engine ops only start at partitions 0/32/64/96, so rows at
                # partitions 3 and 4 are placed with SBUF->SBUF DMA)
                reft = p01.tile([3, S], F32, tag="reft")
                nc.sync.dma_start(reft[:], refT[:])
                sq = p01.tile([3, S], F32, tag="sq")
                nc.vector.tensor_mul(sq[:], reft[:], reft[:])
                n_ps = psA.tile([1, S], F32, tag="psA", name="n_ps")
                for c in range(2):
                    nc.tensor.matmul(n_ps[:, ts(c, 512)], ones3[:], sq[:, ts(c, 512)],
                                     start=True, stop=True)
                n_sb = p01.tile([1, S], F32, tag="n_sb")
                nc.vector.tensor_copy(n_sb[:], n_ps[0:1, :])
                ones_f = p01.tile([1, S], F32, tag="ones_f")
                nc.vector.memset(ones_f[:], 1.0)
                R5 = p01.tile([5, S], F32, tag="R5")
                L5 = p01.tile([5, S], F32, tag="L5")
                nc.vector.tensor_copy(R5[0:3, :], reft[:])
                nc.sync.dma_start(R5[3:4, :], ones_f[:])
                nc.sync.dma_start(R5[4:5, :], n_sb[:])
                nc.vector.tensor_scalar_mul(L5[0:3, :], reft[:], -2.0)
                nc.sync.dma_start(L5[3:4, :], n_sb[:])
                nc.sync.dma_start(L5[4:5, :], ones_f[:])

                # all casts alternate DVE/ACT; GPSIMD's sequencer stays free
                # to act as a third input-DMA queue.
                xqb = cast_load(xq, KT, S, "xqb", None)
                wqb = cast_load(wq, KT, HD, "wqb", None)
                xkb = cast_load(xk, KT, S, "xkb", None)
                wkb = cast_load(wk, KT, HD, "wkb", None)
                xvb = cast_load(xv, KT, S, "xvb", None)
                wvb = cast_load(wv, KT, HD, "wvb", None)
                cast_load(wo, MT, E, "wob", nc.gpsimd, dst=wob)

                # small bias rows
                bvr_f = stage.tile([1, HD], F32, tag="stage", name="bvr_f")
                nc.sync.dma_start(bvr_f[:], bvd.rearrange("(a n) -> a n", a=1))
                nc.vector.tensor_copy(bvr[:], bvr_f[:])
                bor_f = stage.tile([1, E], F32, tag="stage", name="bor_f")
                nc.sync.dma_start(bor_f[:], bod.rearrange("(a n) -> a n", a=1))
                nc.vector.tensor_copy(bor[:], bor_f[:])

                for sp in range(NT):
                    d_ps = psA.tile([128, S], F32, tag="psA", name=f"d_ps{sp}")
                    for c in range(2):
                        nc.tensor.matmul(d_ps[:, ts(c, 512)], L5[:, ts(sp, 128)],
                                         R5[:, ts(c, 512)], start=True, stop=True)
                    nc.vector.tensor_scalar(out=dwp[sp][:], in0=d_ps[:], scalar1=1.0,
                                            scalar2=900.0, op0=ALU.max, op1=ALU.min)
                # ---- QKV projections ----
                for m in range(MT):
                    for which in range(2):
                        wb = (wqb, wkb)[which]
                        xb = (xqb, xkb)[which]
                        bias_col = (bqs, bks)[which]
                        dst = (qtb, ktb)[which]
                        p_ps = psA.tile([128, S], F32, tag="psA",
                                        name=f"p_ps{which}{m}")
                        for c in range(2):
                            for k in range(KT):
                                nc.tensor.matmul(p_ps[:, ts(c, 512)],
                                                 wb[k][:, ts(m, 128)],
                                                 xb[k][:, ts(c, 512)],
                                                 start=(k == 0), stop=(k == KT - 1))
                        nc.scalar.activation(dst[m][:], p_ps[:], AF.Identity,
                                             bias=bias_col[:, m:m + 1])

                for sp in range(NT):
                    v_ps = psA.tile([128, S], F32, tag="psA", name=f"v_ps{sp}")
                    for k in range(KT):
                        nc.tensor.matmul(v_ps[:, 0:HD], xvb[k][:, ts(sp, 128)],
                                         wvb[k][:], start=(k == 0), stop=False)
                    nc.tensor.matmul(v_ps[:, 0:HD], ones_row[:], bvr[:],
                                     start=False, stop=True)
                    v3 = vpb[sp].rearrange("p (h c) -> p h c", c=65)
                    nc.vector.tensor_copy(
                        v3[:, :, 0:64],
                        v_ps[:, 0:HD].rearrange("p (h d) -> p h d", d=64))
                    nc.vector.memset(v3[:, :, 64:65], 1.0)



                # batched by table set: all Ln, then all Exp (2 loads)
                for sp in range(NT):
                    nc.scalar.activation(dwp[sp][:], dwp[sp][:], AF.Ln)
                for sp in range(NT):
                    nc.scalar.activation(dwp[sp][:], dwp[sp][:], AF.Exp,
                                         scale=-0.5, bias=lnb[:])

            # ================= phase 2: per-head attention ===================
            with tc.tile_pool(name="et", bufs=1) as etp:
                for h in range(HG):
                    km = h // 2
                    ro = (h % 2) * 64
                    # et_big[j] holds E^T for sp = 2j, 2j+1 -> one exp per 2 sp
                    et_big = [etp.tile([128, 2 * S], BF16, tag=f"et{j}", bufs=2,
                                       name=f"et{h}_{j}") for j in range(NT // 2)]

                    def et_sl(sp, col, width):
                        return et_big[sp // 2][:, (sp % 2) * S + col:
                                               (sp % 2) * S + col + width]

                    u_ps = psU.tile([65, S], F32, tag="psU", name=f"u_ps{h}")
                    for sp in range(NT):
                        s_ps = psA.tile([128, S], F32, tag="psA",
                                        name=f"s_ps{h}_{sp}")
                        for c in range(2):
                            nc.tensor.matmul(s_ps[:, ts(c, 512)],
                                             ktb[km][ro:ro + 64, ts(sp, 128)],
                                             qtb[km][ro:ro + 64, ts(c, 512)],
                                             start=True, stop=True)
                        # tmod in place in PSUM; exp reads PSUM directly
                        nc.vector.tensor_tensor(out=s_ps[:], in0=s_ps[:],
                                                in1=dwp[sp][:], op=ALU.mult)
                        nc.scalar.activation(et_sl(sp, 0, S), s_ps[:], AF.Exp)
                        # PV accumulation interleaved per sp
                        for c in range(2):
                            nc.tensor.matmul(u_ps[:, ts(c, 512)],
                                             vpb[sp][:, h * 65:(h + 1) * 65],
                                             et_sl(sp, c * 512, 512),
                                             start=(sp == 0), stop=(sp == NT - 1))
                    ub = ubp.tile([65, S], F32, tag="ub", name=f"ub{h}")
                    nc.vector.tensor_copy(ub[:], u_ps[:])

                    for tt in range(NT):
                        up_ps = psB.tile([128, 65], F32, tag="psB", name=f"up{h}_{tt}")
                        nc.tensor.transpose(up_ps[:], ub[:, ts(tt, 128)],
                                            ident_f[0:65, 0:65])
                        rz = tmp.tile([128, 1], F32, tag="rz", bufs=4,
                                      name=f"rz{h}_{tt}")
                        nc.vector.reciprocal(rz[:], up_ps[:, 64:65])
                        o_sb = tmp.tile([128, 64], BF16, tag="osb", bufs=4,
                                        name=f"osb{h}_{tt}")
                        nc.scalar.activation(o_sb[:], up_ps[:, 0:64], AF.Copy,
                                             scale=rz[:])
                        ot_ps = psB.tile([64, 128], BF16, tag="psB",
                                         name=f"otp{h}_{tt}")
                        nc.tensor.transpose(ot_ps[:], o_sb[:], ident[:])
                        nc.scalar.copy(otb[km][ro:ro + 64, ts(tt, 128)], ot_ps[:])

                        p_ps = psB.tile([128, S], BF16, tag="psB",
                                        name=f"pp{h}_{tt}")
                        for sp in range(NT):
                            nc.tensor.transpose(p_ps[:, ts(sp, 128)],
                                                et_sl(sp, tt * 128, 128), ident[:])
                        pt = pstp.tile([128, S], F32, tag="pst", name=f"pt{h}_{tt}")
                        if (h * NT + tt) % 2 == 0:
                            nc.vector.tensor_scalar(out=pt[:], in0=p_ps[:],
                                                    scalar1=rz[:], scalar2=None,
                                                    op0=ALU.mult)
                        else:
                            nc.scalar.activation(pt[:], p_ps[:], AF.Copy,
                                                 scale=rz[:])
                        out_eng = (nc.sync, nc.gpsimd)[(h * NT + tt) % 2]
                        out_eng.dma_start(probs_d[h, ts(tt, 128), :], pt[:])

                # ---- output projection ----
                for tt in range(NT):
                    o_ps = psA.tile([128, S], F32, tag="psA", name=f"o_ps{tt}")
                    for c in range(2):
                        for m in range(MT):
                            nc.tensor.matmul(o_ps[:, ts(c, 512)],
                                             otb[m][:, ts(tt, 128)],
                                             wob[m][:, ts(c, 512)],
                                             start=(m == 0), stop=False)
                        nc.tensor.matmul(o_ps[:, ts(c, 512)], ones_row[:],
                                         bor[:, ts(c, 512)], start=False, stop=True)
                    osb = pstp.tile([128, S], F32, tag="pst", name=f"ov{tt}")
                    if tt % 2 == 0:
                        nc.scalar.copy(osb[:], o_ps[:])
                    else:
                        nc.vector.tensor_copy(osb[:], o_ps[:])
                    nc.sync.dma_start(out_d[ts(tt, 128), :], osb[:])

    nc.compile()
    return nc


_NC = None


def _get_nc():
    global _NC
    if _NC is None:
        _NC = build_module()
    return _NC


def shard_inputs(query, key, value, coords, Wq, bq, Wk, bk, Wv, bv, Wo, bo):
    """Build the 8 per-core input maps (pure slicing / layout, no math)."""
    in_maps = []
    WqT, WkT, WvT, WoT = Wq.T, Wk.T, Wv.T, Wo.T
    bo_half = (bo * 0.5).astype(np.float32)
    for c in range(8):
        b, hg = c // 2, c % 2
        sl = slice(hg * HD, (hg + 1) * HD)
        in_maps.append({
            "xq": np.ascontiguousarray(query[b].T),
            "xk": np.ascontiguousarray(key[b].T),
            "xv": np.ascontiguousarray(value[b].T),
            "refT": np.ascontiguousarray(coords[b, :, 2, :].T),
            "wq": np.ascontiguousarray(WqT[:, sl]),
            "wk": np.ascontiguousarray(WkT[:, sl]),
            "wv": np.ascontiguousarray(WvT[:, sl]),
            "wo": np.ascontiguousarray(WoT[sl, :]),
            "bq": np.ascontiguousarray(bq[sl]),
            "bk": np.ascontiguousarray(bk[sl]),
            "bv": np.ascontiguousarray(bv[sl]),
            "bo": bo_half,
        })
    return in_maps


def kernel(query, key, value, coords, Wq, bq, Wk, bk, Wv, bv, Wo, bo, _trace=False):
    args = [np.asarray(a, np.float32) for a in
            (query, key, value, coords, Wq, bq, Wk, bk, Wv, bv, Wo, bo)]
    nc = _get_nc()
    in_maps = shard_inputs(*args)
    res = run_bass_kernel_spmd(nc, in_maps, core_ids=list(range(8)), trace=_trace)
    B = query.shape[0]
    out = np.zeros((B, S, E), np.float32)
    probs = np.zeros((B, 2 * HG, S, S), np.float32)
    for c in range(8):
        b, hg = c // 2, c % 2
        out[b] += res.results[c]["out_part"]
        probs[b, hg * HG:(hg + 1) * HG] = res.results[c]["probs_part"]
    kernel.last_exec_time_ns = res.exec_time_ns
    kernel.last_results = res
    return out, probs


# revision 29
# speedup vs baseline: 1.0942x; 1.0112x over previous
"""Distance-modulated attention on 8 Trainium2 NeuronCores (Bass/Tile).

Sharding: core c handles batch b = c//2 and head-group hg = c%2 (8 of 16 heads).
Tensor-parallel over heads for QKV/out projections; per-batch distance matrix
is computed on-device per core. Host only slices/transposes (layout), concats,
and sums the two head-group partial outputs per batch (the TP all-reduce).

Per-core device pipeline (S=1024, E=1024, 8 heads x D=64):
  - cast inputs/weights fp32->bf16 on DVE/ACT/GPSIMD
  - d2 via K=5 fp32 matmul of [n_i,1,-2r] x [1,n_j,r]; dw' = 0.625/clip(sqrt(d2),1,30)
    computed as exp(-0.5*ln(clip(d2,1,900)) + ln(0.625)) on ACT (one table set)
  - QT = Wq'^T X^T, KT likewise (transposed layout), V = X Wv' (normal layout,
    bias via K=1 ones-row matmul), all bf16 MMs with fp32 PSUM accumulation
  - per head: S^T = K Q^T (K=64 bf16 MM) -> tmod = S^T * dw' (DVE fp32)
    -> E^T = exp(tmod) (ACT, bf16 out) -> U'^T = [V|1]^T E^T (PV matmul;
    row 64 = softmax denominator Z) -> per 128-row tile: PE-transpose U',
    rz = 1/Z (DVE reciprocal), O = U/Z (bf16), PE-transpose back to O^T;
    probs: PE-transpose E^T tiles, evict*rz to fp32 (DVE/ACT alternating), DMA
  - out = O^T.T Wo' + bo/2 (bf16 MM, bias via ones-row matmul)
"""
import numpy as np
import concourse.bass as bass
import concourse.tile as tile
import concourse.mybir as mybir
from concourse import bacc
from concourse.bass import ts
from concourse.bass_utils import run_bass_kernel_spmd
from concourse.masks import make_identity

F32 = mybir.dt.float32
BF16 = mybir.dt.bfloat16
AF = mybir.ActivationFunctionType
ALU = mybir.AluOpType

S = 1024          # sequence length
E = 1024          # embed dim
HG = 8            # heads per core
D = 64            # head dim
HD = HG * D       # 512, per-core projection width
NT = S // 128     # 8 sequence tiles
KT = E // 128     # 8 contraction tiles
MT = HD // 128    # 4 projection row tiles
LN_B = float(np.log(0.625))   # ln(TEMPERATURE * scaling) = ln(5/8)


def build_module():
    nc = bacc.Bacc(None, target_bir_lowering=False)

    xq = nc.dram_tensor("xq", [E, S], F32, kind="ExternalInput")   # query[b].T
    xk = nc.dram_tensor("xk", [E, S], F32, kind="ExternalInput")
    xv = nc.dram_tensor("xv", [E, S], F32, kind="ExternalInput")
    refT = nc.dram_tensor("refT", [3, S], F32, kind="ExternalInput")
    wq = nc.dram_tensor("wq", [E, HD], F32, kind="ExternalInput")  # Wq.T slice
    wk = nc.dram_tensor("wk", [E, HD], F32, kind="ExternalInput")
    wv = nc.dram_tensor("wv", [E, HD], F32, kind="ExternalInput")
    wo = nc.dram_tensor("wo", [HD, E], F32, kind="ExternalInput")  # Wo.T slice
    bqd = nc.dram_tensor("bq", [HD], F32, kind="ExternalInput")
    bkd = nc.dram_tensor("bk", [HD], F32, kind="ExternalInput")
    bvd = nc.dram_tensor("bv", [HD], F32, kind="ExternalInput")
    bod = nc.dram_tensor("bo", [E], F32, kind="ExternalInput")     # bo * 0.5
    out_d = nc.dram_tensor("out_part", [S, E], F32, kind="ExternalOutput")
    probs_d = nc.dram_tensor("probs_part", [HG, S, S], F32, kind="ExternalOutput")

    with tile.TileContext(nc) as tc:
        with (
            tc.tile_pool(name="const", bufs=1) as constp,
            tc.tile_pool(name="persist", bufs=1) as pers,
            tc.tile_pool(name="tmp", bufs=1) as tmp,
            tc.tile_pool(name="ub", bufs=1) as ubp,
            tc.tile_pool(name="pst", bufs=2) as pstp,
            tc.tile_pool(name="psA", bufs=2, space="PSUM") as psA,
            tc.tile_pool(name="psB", bufs=2, space="PSUM") as psB,
            tc.tile_pool(name="psU", bufs=1, space="PSUM") as psU,
        ):
            # ---- constants ----
            ident = constp.tile([128, 128], BF16, tag="ident")
            make_identity(nc, ident[:])
            ident_f = constp.tile([128, 128], F32, tag="ident_f")
            make_identity(nc, ident_f[:])
            ones_row = constp.tile([1, 128], BF16, tag="ones_row")
            nc.vector.memset(ones_row[:], 1.0)
            ones3 = constp.tile([3, 1], F32, tag="ones3")
            nc.vector.memset(ones3[:], 1.0)
            lnb = constp.tile([128, 1], F32, tag="lnb")
            nc.vector.memset(lnb[:], LN_B)
            bqs = constp.tile([128, MT], F32, tag="bqs")
            bks = constp.tile([128, MT], F32, tag="bks")
            with nc.allow_non_contiguous_dma("tiny bias loads"):
                nc.sync.dma_start(bqs[:], bqd.rearrange("(m p) -> p m", p=128))
                nc.sync.dma_start(bks[:], bkd.rearrange("(m p) -> p m", p=128))
            bvr = constp.tile([1, HD], BF16, tag="bvr")
            bor = constp.tile([1, E], BF16, tag="bor")

            # persistent activations / weights for later phases
            dwp = [pers.tile([128, S], F32, tag=f"dwp{i}", name=f"dwp{i}")
                   for i in range(NT)]
            qtb = [pers.tile([128, S], BF16, tag=f"qtb{m}", name=f"qtb{m}")
                   for m in range(MT)]
            ktb = [pers.tile([128, S], BF16, tag=f"ktb{m}", name=f"ktb{m}")
                   for m in range(MT)]
            vpb = [pers.tile([128, HG * 65], BF16, tag=f"vpb{i}", name=f"vpb{i}")
                   for i in range(NT)]
            otb = [pers.tile([128, S], BF16, tag=f"otb{m}", name=f"otb{m}")
                   for m in range(MT)]
            wob = [pers.tile([128, E], BF16, tag=f"wob{m}", name=f"wob{m}")
                   for m in range(MT)]

            # ================= phase 0/1: loads, dw, projections =============
            with (
                tc.tile_pool(name="p01", bufs=1) as p01,
                tc.tile_pool(name="stage", bufs=4) as stage,
            ):
                dma_engs = [nc.sync, nc.scalar, nc.gpsimd]
                dma_i = [0]

                def cast_load(dram, n_tiles, width, tag, eng, dst=None, dq=None):
                    tiles = []
                    for i in range(n_tiles):
                        st = stage.tile([128, 1024], F32, tag="stage", name=f"st_{tag}{i}")
                        q = dq if dq is not None else dma_engs[dma_i[0] % 3]
                        q.dma_start(st[:, :width], dram[ts(i, 128), :])
                        dma_i[0] += 1
                        if dst is None:
                            bt = p01.tile([128, width], BF16, tag=f"{tag}{i}",
                                          name=f"{tag}{i}")
                        else:
                            bt = dst[i]
                        e = eng if eng is not None else (nc.vector, nc.scalar)[i % 2]
                        if e is nc.scalar:
                            e.copy(bt[:], st[:, :width])
                        else:
                            e.tensor_copy(bt[:], st[:, :width])
                        tiles.append(bt)
                    return tiles

                # ---- distance weights dw' ----
                # 5-dim contraction rows: L = [-2r; n; 1], R = [r; 1; n]
                # (engine ops only start at partitions 0/32/64/96, so rows at
                # partitions 3 and 4 are placed with SBUF->SBUF DMA)
                reft = p01.tile([3, S], F32, tag="reft")
                nc.sync.dma_start(reft[:], refT[:])
                sq = p01.tile([3, S], F32, tag="sq")
                nc.vector.tensor_mul(sq[:], reft[:], reft[:])
                n_ps = psA.tile([1, S], F32, tag="psA", name="n_ps")
                for c in range(2):
                    nc.tensor.matmul(n_ps[:, ts(c, 512)], ones3[:], sq[:, ts(c, 512)],
                                     start=True, stop=True)
                n_sb = p01.tile([1, S], F32, tag="n_sb")
                nc.vector.tensor_copy(n_sb[:], n_ps[0:1, :])
                ones_f = p01.tile([1, S], F32, tag="ones_f")
                nc.vector.memset(ones_f[:], 1.0)
                R5 = p01.tile([5, S], F32, tag="R5")
                L5 = p01.tile([5, S], F32, tag="L5")
                nc.vector.tensor_copy(R5[0:3, :], reft[:])
                nc.sync.dma_start(R5[3:4, :], ones_f[:])
                nc.sync.dma_start(R5[4:5, :], n_sb[:])
                nc.vector.tensor_scalar_mul(L5[0:3, :], reft[:], -2.0)
                nc.sync.dma_start(L5[3:4, :], n_sb[:])
                nc.sync.dma_start(L5[4:5, :], ones_f[:])

                # all casts alternate DVE/ACT; GPSIMD's sequencer stays free
                # to act as a third input-DMA queue.
                xqb = cast_load(xq, KT, S, "xqb", None)
                wqb = cast_load(wq, KT, HD, "wqb", None)
                xkb = cast_load(xk, KT, S, "xkb", None)
                wkb = cast_load(wk, KT, HD, "wkb", None)
                xvb = cast_load(xv, KT, S, "xvb", None)
                wvb = cast_load(wv, KT, HD, "wvb", None)
                cast_load(wo, MT, E, "wob", nc.gpsimd, dst=wob)

                # small bias rows
                bvr_f = stage.tile([1, HD], F32, tag="stage", name="bvr_f")
                nc.sync.dma_start(bvr_f[:], bvd.rearrange("(a n) -> a n", a=1))
                nc.vector.tensor_copy(bvr[:], bvr_f[:])
                bor_f = stage.tile([1, E], F32, tag="stage", name="bor_f")
                nc.sync.dma_start(bor_f[:], bod.rearrange("(a n) -> a n", a=1))
                nc.vector.tensor_copy(bor[:], bor_f[:])

                for sp in range(NT):
                    d_ps = psA.tile([128, S], F32, tag="psA", name=f"d_ps{sp}")
                    for c in range(2):
                        nc.tensor.matmul(d_ps[:, ts(c, 512)], L5[:, ts(sp, 128)],
                                         R5[:, ts(c, 512)], start=True, stop=True)
                    nc.vector.tensor_scalar(out=dwp[sp][:], in0=d_ps[:], scalar1=1.0,
                                            scalar2=900.0, op0=ALU.max, op1=ALU.min)
                # ---- QKV projections ----
                for m in range(MT):
                    for which in range(2):
                        wb = (wqb, wkb)[which]
                        xb = (xqb, xkb)[which]
                        bias_col = (bqs, bks)[which]
                        dst = (qtb, ktb)[which]
                        p_ps = psA.tile([128, S], F32, tag="psA",
                                        name=f"p_ps{which}{m}")
                        for c in range(2):
                            for k in range(KT):
                                nc.tensor.matmul(p_ps[:, ts(c, 512)],
                                                 wb[k][:, ts(m, 128)],
                                                 xb[k][:, ts(c, 512)],
                                                 start=(k == 0), stop=(k == KT - 1))
                        nc.scalar.activation(dst[m][:], p_ps[:], AF.Identity,
                                             bias=bias_col[:, m:m + 1])

                for sp in range(NT):
                    v_ps = psA.tile([128, S], F32, tag="psA", name=f"v_ps{sp}")
                    for k in range(KT):
                        nc.tensor.matmul(v_ps[:, 0:HD], xvb[k][:, ts(sp, 128)],
                                         wvb[k][:], start=(k == 0), stop=False)
                    nc.tensor.matmul(v_ps[:, 0:HD], ones_row[:], bvr[:],
                                     start=False, stop=True)
                    v3 = vpb[sp].rearrange("p (h c) -> p h c", c=65)
                    nc.vector.tensor_copy(
                        v3[:, :, 0:64],
                        v_ps[:, 0:HD].rearrange("p (h d) -> p h d", d=64))
                    nc.vector.memset(v3[:, :, 64:65], 1.0)



                # batched by table set: all Ln, then all Exp (2 loads)
                for sp in range(NT):
                    nc.scalar.activation(dwp[sp][:], dwp[sp][:], AF.Ln)
                for sp in range(NT):
                    nc.scalar.activation(dwp[sp][:], dwp[sp][:], AF.Exp,
                                         scale=-0.5, bias=lnb[:])

            # ================= phase 2: per-head attention ===================
            with tc.tile_pool(name="et", bufs=1) as etp:
                for h in range(HG):
                    km = h // 2
                    ro = (h % 2) * 64
                    # et_big[j] holds E^T for sp = 2j, 2j+1 -> one exp per 2 sp
                    et_big = [etp.tile([128, 2 * S], BF16, tag=f"et{j}", bufs=2,
                                       name=f"et{h}_{j}") for j in range(NT // 2)]

                    def et_sl(sp, col, width):
                        return et_big[sp // 2][:, (sp % 2) * S + col:
                                               (sp % 2) * S + col + width]

                    u_ps = psU.tile([65, S], F32, tag="psU", name=f"u_ps{h}")
                    for sp in range(NT):
                        s_ps = psA.tile([128, S], F32, tag="psA",
                                        name=f"s_ps{h}_{sp}")
                        for c in range(2):
                            nc.tensor.matmul(s_ps[:, ts(c, 512)],
                                             ktb[km][ro:ro + 64, ts(sp, 128)],
                                             qtb[km][ro:ro + 64, ts(c, 512)],
                                             start=True, stop=True)
                        # tmod in place in PSUM; exp reads PSUM directly
                        nc.vector.tensor_tensor(out=s_ps[:], in0=s_ps[:],
                                                in1=dwp[sp][:], op=ALU.mult)
                        nc.scalar.activation(et_sl(sp, 0, S), s_ps[:], AF.Exp)
                        # PV accumulation interleaved per sp
                        for c in range(2):
                            nc.tensor.matmul(u_ps[:, ts(c, 512)],
                                             vpb[sp][:, h * 65:(h + 1) * 65],
                                             et_sl(sp, c * 512, 512),
                                             start=(sp == 0), stop=(sp == NT - 1))
                    ub = ubp.tile([65, S], F32, tag="ub", name=f"ub{h}")
                    nc.vector.tensor_copy(ub[:], u_ps[:])

                    for tt in range(NT):
                        up_ps = psB.tile([128, 65], F32, tag="psB", name=f"up{h}_{tt}")
                        nc.tensor.transpose(up_ps[:], ub[:, ts(tt, 128)],
                                            ident_f[0:65, 0:65])
                        rz = tmp.tile([128, 1], F32, tag="rz", bufs=4,
                                      name=f"rz{h}_{tt}")
                        nc.vector.reciprocal(rz[:], up_ps[:, 64:65])
                        o_sb = tmp.tile([128, 64], BF16, tag="osb", bufs=4,
                                        name=f"osb{h}_{tt}")
                        nc.scalar.activation(o_sb[:], up_ps[:, 0:64], AF.Copy,
                                             scale=rz[:])
                        ot_ps = psB.tile([64, 128], BF16, tag="psB",
                                         name=f"otp{h}_{tt}")
                        nc.tensor.transpose(ot_ps[:], o_sb[:], ident[:])
                        nc.scalar.copy(otb[km][ro:ro + 64, ts(tt, 128)], ot_ps[:])

                        p_ps = psB.tile([128, S], BF16, tag="psB",
                                        name=f"pp{h}_{tt}")
                        for sp in range(NT):
                            nc.tensor.transpose(p_ps[:, ts(sp, 128)],
                                                et_sl(sp, tt * 128, 128), ident[:])
                        pt = pstp.tile([128, S], F32, tag="pst", name=f"pt{h}_{tt}")
                        if (h * NT + tt) % 3 == 0:
                            nc.vector.tensor_scalar(out=pt[:], in0=p_ps[:],
                                                    scalar1=rz[:], scalar2=None,
                                                    op0=ALU.mult)
                        else:
                            nc.scalar.activation(pt[:], p_ps[:], AF.Copy,
                                                 scale=rz[:])
                        out_eng = (nc.sync, nc.gpsimd)[(h * NT + tt) % 2]
                        out_eng.dma_start(probs_d[h, ts(tt, 128), :], pt[:])

                # ---- output projection ----
                for tt in range(NT):
                    o_ps = psA.tile([128, S], F32, tag="psA", name=f"o_ps{tt}")
                    for c in range(2):
                        for m in range(MT):
                            nc.tensor.matmul(o_ps[:, ts(c, 512)],
                                             otb[m][:, ts(tt, 128)],
                                             wob[m][:, ts(c, 512)],
                                             start=(m == 0), stop=False)
                        nc.tensor.matmul(o_ps[:, ts(c, 512)], ones_row[:],
                                         bor[:, ts(c, 512)], start=False, stop=True)
                    osb = pstp.tile([128, S], F32, tag="pst", name=f"ov{tt}")
                    if tt % 2 == 0:
                        nc.scalar.copy(osb[:], o_ps[:])
                    else:
                        nc.vector.tensor_copy(osb[:], o_ps[:])
                    nc.sync.dma_start(out_d[ts(tt, 128), :], osb[:])

    nc.compile()
    return nc


_NC = None


def _get_nc():
    global _NC
    if _NC is None:
        _NC = build_module()
    return _NC


def shard_inputs(query, key, value, coords, Wq, bq, Wk, bk, Wv, bv, Wo, bo):
    """Build the 8 per-core input maps (pure slicing / layout, no math)."""
    in_maps = []
    WqT, WkT, WvT, WoT = Wq.T, Wk.T, Wv.T, Wo.T
    bo_half = (bo * 0.5).astype(np.float32)
    for c in range(8):
        b, hg = c // 2, c % 2
        sl = slice(hg * HD, (hg + 1) * HD)
        in_maps.append({
            "xq": np.ascontiguousarray(query[b].T),
            "xk": np.ascontiguousarray(key[b].T),
            "xv": np.ascontiguousarray(value[b].T),
            "refT": np.ascontiguousarray(coords[b, :, 2, :].T),
            "wq": np.ascontiguousarray(WqT[:, sl]),
            "wk": np.ascontiguousarray(WkT[:, sl]),
            "wv": np.ascontiguousarray(WvT[:, sl]),
            "wo": np.ascontiguousarray(WoT[sl, :]),
            "bq": np.ascontiguousarray(bq[sl]),
            "bk": np.ascontiguousarray(bk[sl]),
            "bv": np.ascontiguousarray(bv[sl]),
            "bo": bo_half,
        })
    return in_maps


def kernel(query, key, value, coords, Wq, bq, Wk, bk, Wv, bv, Wo, bo, _trace=False):
    args = [np.asarray(a, np.float32) for a in
            (query, key, value, coords, Wq, bq, Wk, bk, Wv, bv, Wo, bo)]
    nc = _get_nc()
    in_maps = shard_inputs(*args)
    res = run_bass_kernel_spmd(nc, in_maps, core_ids=list(range(8)), trace=_trace)
    B = query.shape[0]
    out = np.zeros((B, S, E), np.float32)
    probs = np.zeros((B, 2 * HG, S, S), np.float32)
    for c in range(8):
        b, hg = c // 2, c % 2
        out[b] += res.results[c]["out_part"]
        probs[b, hg * HG:(hg + 1) * HG] = res.results[c]["probs_part"]
    kernel.last_exec_time_ns = res.exec_time_ns
    kernel.last_results = res
    return out, probs
